# revision 20
# baseline (speedup 1.0000x reference)
"""Trainium2 Bass kernel for nn_DecoderLayer (post-LN decoder layer).

Sharding: data-parallel over batch. B=8 batch elements -> 8 NeuronCores,
one full decoder layer per core, zero collectives.

Per-core layout strategy:
  - Matmul inputs are feature-major (D on partitions): xT, memT, WT (host
    pre-transposed, bf16).
  - Residual stream + LayerNorm are token-major (tokens on partitions) so
    LN reductions run along the free dim and softmax-normalization is a
    per-partition scalar op.
  - Attention per head h: scoresT[m,n] = k_h^T q_h (lhsT=kT_h, K=64),
    exp on ScalarE (scale=1/8 fused, no max subtraction -- scores are O(1)
    here), then o[n,:] = lhsT(expT chunk)^T @ [v_h | ones]; column 64 gives
    sum(exp) so normalization is one reciprocal + per-partition multiply
    on VectorE. ScalarE runs *only* exp (+2 small LN ops): all PSUM
    evictions are pinned to VectorE.
  - LayerNorm is two-pass: residual+bn_stats per chunk, then one batched
    Ln/Exp pair for all 8 chunks' rstd (avoids ACT table-set thrashing),
    then the apply pass. x1/x2 stay resident in SBUF; the feature-major
    copies are made with 64 SBUF->SBUF xbar DMA transposes.
  - FFN is chunked over FF (4 x 1024) with an SBUF f32 accumulator that is
    initialized with the residual input, so LN3 reads it directly.
  - Host folds biases: x_tok = x + bv1; be1' = be1 + bv2; bq2' = bq2 -
    Wq2 @ bv2; be2' = be2 + b2; b1' = b1 - W1 @ b2.  bq/bk/b1 are applied
    on-device as per-partition bias during PSUM eviction. When g==1 and
    be'==0 (checked at runtime) a variant without the g/be multiplies is
    used.
"""

import os
import numpy as np
import ml_dtypes

BF16 = ml_dtypes.bfloat16

D = 1024
N = 1024
H = 16
DK = 64
FF = 4096
P = 128
DC = D // P     # 8 feature chunks
NC_ = N // P    # 8 token chunks
EPS = 1e-5
N_CORES = 8

_BUILD_CACHE = {}


def _build_program(loop_n=1, ln_identity=False):
    """Build the Bass/Tile program. loop_n > 1 replicates the layer body
    (timing only). ln_identity skips the LN gamma/beta applies."""
    key = (loop_n, ln_identity)
    if key in _BUILD_CACHE:
        return _BUILD_CACHE[key]

    from contextlib import ExitStack

    import concourse.bass as bass
    import concourse.mybir as mybir
    import concourse.tile as tile
    from concourse import bacc
    from concourse.masks import make_identity

    dt = mybir.dt
    AF = mybir.ActivationFunctionType
    ALU = mybir.AluOpType

    nc = bacc.Bacc("TRN2", target_bir_lowering=False, debug=False)

    # ---- DRAM parameters (per core) ----
    x_tok_d = nc.dram_tensor("x_tok", [N, D], dt.bfloat16, kind="ExternalInput")
    xT_d = nc.dram_tensor("xT", [D, N], dt.bfloat16, kind="ExternalInput")
    memT_d = nc.dram_tensor("memT", [D, N], dt.bfloat16, kind="ExternalInput")
    WqT1_d = nc.dram_tensor("WqT1", [D, D], dt.bfloat16, kind="ExternalInput")
    WkT1_d = nc.dram_tensor("WkT1", [D, D], dt.bfloat16, kind="ExternalInput")
    WvT1_d = nc.dram_tensor("WvT1", [D, D], dt.bfloat16, kind="ExternalInput")
    WqT2_d = nc.dram_tensor("WqT2", [D, D], dt.bfloat16, kind="ExternalInput")
    WkT2_d = nc.dram_tensor("WkT2", [D, D], dt.bfloat16, kind="ExternalInput")
    WvT2_d = nc.dram_tensor("WvT2", [D, D], dt.bfloat16, kind="ExternalInput")
    W1T_d = nc.dram_tensor("W1T", [D, FF], dt.bfloat16, kind="ExternalInput")
    W2T_d = nc.dram_tensor("W2T", [FF, D], dt.bfloat16, kind="ExternalInput")
    bq1_d = nc.dram_tensor("bq1", [D], dt.float32, kind="ExternalInput")
    bk1_d = nc.dram_tensor("bk1", [D], dt.float32, kind="ExternalInput")
    bq2_d = nc.dram_tensor("bq2p", [D], dt.float32, kind="ExternalInput")
    bk2_d = nc.dram_tensor("bk2", [D], dt.float32, kind="ExternalInput")
    b1_d = nc.dram_tensor("b1p", [FF], dt.float32, kind="ExternalInput")
    g1_d = nc.dram_tensor("g1", [D], dt.bfloat16, kind="ExternalInput")
    be1_d = nc.dram_tensor("be1p", [D], dt.bfloat16, kind="ExternalInput")
    g2_d = nc.dram_tensor("g2", [D], dt.bfloat16, kind="ExternalInput")
    be2_d = nc.dram_tensor("be2p", [D], dt.bfloat16, kind="ExternalInput")
    g3_d = nc.dram_tensor("g3", [D], dt.bfloat16, kind="ExternalInput")
    be3_d = nc.dram_tensor("be3", [D], dt.bfloat16, kind="ExternalInput")
    out_d = nc.dram_tensor("out", [N, D], dt.float32, kind="ExternalOutput")

    def bcast_ap(handle, n):
        return bass.AP(tensor=handle, offset=0, ap=[[0, P], [1, n]])

    def colmajor_ap(handle, chunks):
        # [chunks*P] f32 vector -> SBUF [P, chunks] with [p, c] = v[c*P + p]
        return bass.AP(tensor=handle, offset=0, ap=[[1, P], [P, chunks]])

    with tile.TileContext(nc) as tc, ExitStack() as top:
        consts = top.enter_context(tc.tile_pool(name="consts", bufs=1))
        wpool = top.enter_context(tc.tile_pool(name="w", bufs=2))
        xTp = top.enter_context(tc.tile_pool(name="xT", bufs=1))
        opool = top.enter_context(tc.tile_pool(name="o", bufs=1))
        xsb = top.enter_context(tc.tile_pool(name="xsb", bufs=2))
        small = top.enter_context(tc.tile_pool(name="small", bufs=10))

        # ---- constants ----
        bq1_sb = consts.tile([P, DC], dt.float32, tag="c_bq1")
        nc.sync.dma_start(out=bq1_sb, in_=colmajor_ap(bq1_d, DC))
        bk1_sb = consts.tile([P, DC], dt.float32, tag="c_bk1")
        nc.sync.dma_start(out=bk1_sb, in_=colmajor_ap(bk1_d, DC))
        bq2_sb = consts.tile([P, DC], dt.float32, tag="c_bq2")
        nc.sync.dma_start(out=bq2_sb, in_=colmajor_ap(bq2_d, DC))
        bk2_sb = consts.tile([P, DC], dt.float32, tag="c_bk2")
        nc.sync.dma_start(out=bk2_sb, in_=colmajor_ap(bk2_d, DC))
        b1_sb = consts.tile([P, FF // P], dt.float32, tag="c_b1")
        nc.sync.dma_start(out=b1_sb, in_=colmajor_ap(b1_d, FF // P))
        eps_sb = consts.tile([P, 1], dt.float32, tag="c_eps")
        nc.vector.memset(eps_sb, EPS)
        ident_sb = consts.tile([P, P], dt.bfloat16, tag="c_ident")
        make_identity(nc, ident_sb)

        def load_w(dram_h, row0=0, nrows=None, col0=0, ncolw=D):
            """Stream rows [row0, row0+nrows) cols [col0, col0+ncolw) of a
            transposed weight into SBUF [P, nrows//P, ncolw], split per
            k-chunk so consuming matmuls start as soon as chunk 0 lands."""
            rows, ncols = dram_h.shape
            if nrows is None:
                nrows = rows
            w = wpool.tile([P, nrows // P, ncolw], dt.bfloat16, tag="w",
                           name=f"w_{dram_h.name}_{row0}_{col0}")
            for kc in range(nrows // P):
                nc.sync.dma_start(
                    out=w[:, kc, :],
                    in_=bass.AP(tensor=dram_h,
                                offset=(row0 + kc * P) * ncols + col0,
                                ap=[[ncols, P], [1, ncolw]]),
                )
            return w

        def proj_fm(out_sb, wT_sb, inT_sb, bias_sb, pp):
            """Feature-major projection: out[o,n] = sum_d WT[d,o] inT[d,n] + b[o]."""
            for dc in range(DC):
                for nt in range(2):
                    ps = pp.tile([P, 512], dt.float32, tag="pp")
                    for kc in range(DC):
                        nc.tensor.matmul(
                            ps,
                            lhsT=wT_sb[:, kc, dc * P:(dc + 1) * P],
                            rhs=inT_sb[:, kc, nt * 512:(nt + 1) * 512],
                            start=(kc == 0),
                            stop=(kc == DC - 1),
                        )
                    nc.vector.tensor_scalar(
                        out_sb[:, dc, nt * 512:(nt + 1) * 512],
                        ps, bias_sb[:, dc:dc + 1], None, ALU.add,
                    )

        def proj_v65(v65_sb, wT_sb, inT_sb, pp):
            """Token-major V projection into [P, mc, h, 0:64]; col 64 stays 1.0."""
            nc.gpsimd.memset(v65_sb, 1.0)
            for mc in range(NC_):
                for ot in range(2):
                    ps = pp.tile([P, 512], dt.float32, tag="pp")
                    for kc in range(DC):
                        nc.tensor.matmul(
                            ps,
                            lhsT=inT_sb[:, kc, mc * P:(mc + 1) * P],
                            rhs=wT_sb[:, kc, ot * 512:(ot + 1) * 512],
                            start=(kc == 0),
                            stop=(kc == DC - 1),
                        )
                    nc.vector.tensor_copy(
                        v65_sb[:, mc, ot * 8:(ot + 1) * 8, 0:64],
                        ps.rearrange("p (h e) -> p h e", h=8),
                    )

        def attention(qT_sb, kT_sb, v65_sb, o_sb, ep, sp, op65):
            for h_idx in range(H):
                hp, half = divmod(h_idx, 2)
                lo = half * 64
                expT = ep.tile([P, NC_, N], dt.bfloat16, tag="exp",
                               name=f"exp_{h_idx}")
                for mc in range(NC_):
                    ps = sp.tile([P, N], dt.float32, tag="sp",
                                 name=f"ps_{h_idx}_{mc}")
                    for nt in range(2):
                        nc.tensor.matmul(
                            ps[:, nt * 512:(nt + 1) * 512],
                            lhsT=kT_sb[lo:lo + 64, hp, mc * P:(mc + 1) * P],
                            rhs=qT_sb[lo:lo + 64, hp, nt * 512:(nt + 1) * 512],
                            start=True, stop=True,
                        )
                    nc.scalar.activation(
                        expT[:, mc, :], ps, AF.Exp, scale=0.125,
                    )
                for ncc in range(NC_):
                    po = op65.tile([P, 65], dt.float32, tag="op65")
                    for mc in range(NC_):
                        nc.tensor.matmul(
                            po,
                            lhsT=expT[:, mc, ncc * P:(ncc + 1) * P],
                            rhs=v65_sb[:, mc, h_idx, :],
                            start=(mc == 0),
                            stop=(mc == NC_ - 1),
                        )
                    rec = small.tile([P, 1], dt.float32, tag="rec")
                    nc.vector.reciprocal(rec, po[:, 64:65])
                    nc.vector.tensor_scalar(
                        o_sb[:, ncc, h_idx * 64:(h_idx + 1) * 64],
                        po[:, 0:64], rec, None, ALU.mult,
                    )

        def layernorm(r_sb, x_src, g_d, be_d, dst_sb, lnp, final=False):
            """dst = LN(x_src + r_sb)*g + be, token-major, two passes.

            r_sb: SBUF [P, NC_, D] (attention out / f32 FFN accumulator);
                  mutated in place to hold the residual sum unless x_src
                  is None.
            x_src: DRAM handle (LN1), SBUF [P, NC_, D] tensor (LN2), or
                  None (LN3: r_sb already holds the residual sum).
            dst_sb: SBUF [P, NC_, D] bf16 (x1/x2) or, if final, the DRAM
                  output handle.
            """
            mv = small.tile([P, NC_, 2], dt.float32, tag="mv",
                            name=f"mv_{'f' if final else dst_sb.name}")
            for ncc in range(NC_):
                if x_src is not None:
                    if callable(getattr(x_src, "ap", None)):  # DRAM handle
                        xin = lnp.tile([P, D], dt.bfloat16, tag="lnx")
                        nc.sync.dma_start(
                            out=xin, in_=x_src.ap()[ncc * P:(ncc + 1) * P, :]
                        )
                    else:
                        xin = x_src[:, ncc, :]
                    r = r_sb[:, ncc, :]
                    nc.vector.tensor_tensor(out=r, in0=r, in1=xin, op=ALU.add)
                else:
                    r = r_sb[:, ncc, :]
                stats = small.tile([P, 2, 6], dt.float32, tag="stats")
                nc.vector.bn_stats(stats[:, 0, :], r[:, 0:512])
                nc.vector.bn_stats(stats[:, 1, :], r[:, 512:1024])
                nc.vector.bn_aggr(mv[:, ncc, :], stats)
            # batched rstd = exp(-0.5*ln(var+eps)): one table-load pair per LN
            lnv = small.tile([P, NC_], dt.float32, tag="lnv")
            nc.scalar.activation(lnv, mv[:, :, 1], AF.Ln, bias=eps_sb)
            rstd = small.tile([P, NC_], dt.float32, tag="rstd")
            nc.scalar.activation(rstd, lnv, AF.Exp, scale=-0.5)
            nmr = small.tile([P, NC_], dt.float32, tag="nmr")
            nc.vector.tensor_tensor(out=nmr, in0=mv[:, :, 0], in1=rstd,
                                    op=ALU.mult)
            nc.vector.tensor_scalar(nmr, nmr, -1.0, None, ALU.mult)
            if not ln_identity:
                g_t = lnp.tile([P, D], dt.bfloat16, tag="lng",
                               name=f"g_{g_d.name}")
                nc.sync.dma_start(out=g_t, in_=bcast_ap(g_d, D))
                be_t = lnp.tile([P, D], dt.bfloat16, tag="lnbe",
                                name=f"be_{be_d.name}")
                nc.sync.dma_start(out=be_t, in_=bcast_ap(be_d, D))
            for ncc in range(NC_):
                r = r_sb[:, ncc, :]
                if final:
                    oc = lnp.tile([P, D], dt.float32, tag="lnof")
                    tgt = oc
                else:
                    tgt = dst_sb[:, ncc, :]
                if ln_identity:
                    nc.any.tensor_scalar(
                        tgt, r, rstd[:, ncc:ncc + 1], nmr[:, ncc:ncc + 1],
                        ALU.mult, ALU.add)
                else:
                    t = lnp.tile([P, D], dt.bfloat16, tag="lnt")
                    nc.vector.tensor_scalar(
                        t, r, rstd[:, ncc:ncc + 1], nmr[:, ncc:ncc + 1],
                        ALU.mult, ALU.add)
                    nc.vector.tensor_tensor(out=t, in0=t, in1=g_t, op=ALU.mult)
                    nc.vector.tensor_tensor(out=tgt, in0=t, in1=be_t,
                                            op=ALU.add)
                if final:
                    nc.sync.dma_start(
                        out=out_d.ap()[ncc * P:(ncc + 1) * P, :], in_=tgt
                    )

        def transpose_in(xT_sb, src_sb):
            """64 PE transposes + DVE evictions (PE is idle in LN windows)."""
            with tc.tile_pool(name="tp", bufs=4, space="PSUM") as tpp:
                for ncc in range(NC_):
                    for dc in range(DC):
                        pst = tpp.tile([P, P], dt.bfloat16, tag="tp")
                        nc.tensor.transpose(
                            pst, src_sb[:, ncc, dc * P:(dc + 1) * P], ident_sb)
                        nc.any.tensor_copy(
                            xT_sb[:, dc, ncc * P:(ncc + 1) * P], pst)

        def one_layer():
         with ExitStack() as attn_scope:
            qp = attn_scope.enter_context(tc.tile_pool(name="q", bufs=1))
            kp = attn_scope.enter_context(tc.tile_pool(name="k", bufs=1))
            vp = attn_scope.enter_context(tc.tile_pool(name="v", bufs=1))
            ep = attn_scope.enter_context(tc.tile_pool(name="exp", bufs=2))

            with tc.tile_pool(name="mem", bufs=1) as memp:
                xT0 = xTp.tile([P, DC, N], dt.bfloat16, tag="xT")
                for kc in range(DC):
                    nc.sync.dma_start(
                        out=xT0[:, kc, :],
                        in_=bass.AP(tensor=xT_d, offset=kc * P * N,
                                    ap=[[N, P], [1, N]]))
                memT = memp.tile([P, DC, N], dt.bfloat16, tag="memT")
                for kc in range(DC):
                    nc.sync.dma_start(
                        out=memT[:, kc, :],
                        in_=bass.AP(tensor=memT_d, offset=kc * P * N,
                                    ap=[[N, P], [1, N]]))

                # ---- self-attention QKV ----
                q1T = qp.tile([P, DC, N], dt.bfloat16, tag="qT")
                k1T = kp.tile([P, DC, N], dt.bfloat16, tag="kT")
                v1 = vp.tile([P, NC_, H, 65], dt.bfloat16, tag="v65")
                with tc.tile_pool(name="pp1", bufs=4, space="PSUM") as pp:
                    proj_fm(q1T, load_w(WqT1_d), xT0, bq1_sb, pp)
                    proj_fm(k1T, load_w(WkT1_d), xT0, bk1_sb, pp)
                    proj_v65(v1, load_w(WvT1_d), xT0, pp)

                # ---- self-attention ----
                o1 = opool.tile([P, NC_, D], dt.bfloat16, tag="o")
                with tc.tile_pool(name="sp1", bufs=3, space="PSUM") as sp, \
                     tc.tile_pool(name="ov1", bufs=2, space="PSUM") as op65:
                    attention(q1T, k1T, v1, o1, ep, sp, op65)

                # ---- cross-attention K/V (independent of LN1: fills the
                # LN1 gap with PE work) ----
                k2T = kp.tile([P, DC, N], dt.bfloat16, tag="kT")
                v2 = vp.tile([P, NC_, H, 65], dt.bfloat16, tag="v65")
                with tc.tile_pool(name="pp2", bufs=4, space="PSUM") as pp:
                    proj_fm(k2T, load_w(WkT2_d), memT, bk2_sb, pp)
                    proj_v65(v2, load_w(WvT2_d), memT, pp)

            # ---- LN1 -> x1 (SBUF resident) + x1T ----
            x1_sb = xsb.tile([P, NC_, D], dt.bfloat16, tag="xsb", name="x1sb")
            with tc.tile_pool(name="ln1", bufs=2) as lnp:
                layernorm(o1, x_tok_d, g1_d, be1_d, x1_sb, lnp)
            x1T = xTp.tile([P, DC, N], dt.bfloat16, tag="xT")
            transpose_in(x1T, x1_sb)

            # ---- cross-attention Q ----
            q2T = qp.tile([P, DC, N], dt.bfloat16, tag="qT")
            with tc.tile_pool(name="pp3", bufs=4, space="PSUM") as pp:
                proj_fm(q2T, load_w(WqT2_d), x1T, bq2_sb, pp)

            # ---- cross-attention ----
            o2 = opool.tile([P, NC_, D], dt.bfloat16, tag="o")
            with tc.tile_pool(name="sp2", bufs=3, space="PSUM") as sp, \
                 tc.tile_pool(name="ov2", bufs=2, space="PSUM") as op65:
                attention(q2T, k2T, v2, o2, ep, sp, op65)

            # ---- LN2 -> x2 (SBUF resident) + x2T ----
            x2_sb = xsb.tile([P, NC_, D], dt.bfloat16, tag="xsb", name="x2sb")
            with tc.tile_pool(name="ln2", bufs=2) as lnp:
                layernorm(o2, x1_sb, g2_d, be2_d, x2_sb, lnp)
            x2T = xTp.tile([P, DC, N], dt.bfloat16, tag="xT")
            transpose_in(x2T, x2_sb)

         # ---- FFN ----
         with ExitStack() as ffn_scope:
            hp_ = ffn_scope.enter_context(tc.tile_pool(name="h", bufs=2))
            yp = ffn_scope.enter_context(tc.tile_pool(name="y", bufs=1))
            lnp3 = ffn_scope.enter_context(tc.tile_pool(name="ln3", bufs=2))
            ppf = ffn_scope.enter_context(
                tc.tile_pool(name="ppf", bufs=4, space="PSUM"))
            y_acc = yp.tile([P, NC_, D], dt.float32, tag="y")
            for f in range(FF // D):
                w1f = load_w(W1T_d, col0=f * D)
                hf = hp_.tile([P, DC, N], dt.bfloat16, tag="h")
                for fc in range(DC):
                    for nt in range(2):
                        ps = ppf.tile([P, 512], dt.float32, tag="pp")
                        for kc in range(DC):
                            nc.tensor.matmul(
                                ps,
                                lhsT=w1f[:, kc, fc * P:(fc + 1) * P],
                                rhs=x2T[:, kc, nt * 512:(nt + 1) * 512],
                                start=(kc == 0),
                                stop=(kc == DC - 1),
                            )
                        # h = relu(psum + b1')
                        nc.vector.tensor_scalar(
                            hf[:, fc, nt * 512:(nt + 1) * 512],
                            ps, b1_sb[:, f * DC + fc:f * DC + fc + 1], 0.0,
                            ALU.add, ALU.max,
                        )
                w2f = load_w(W2T_d, row0=f * D, nrows=D)
                for ncc in range(NC_):
                    for dtile in range(2):
                        ps = ppf.tile([P, 512], dt.float32, tag="pp")
                        for fc in range(DC):
                            nc.tensor.matmul(
                                ps,
                                lhsT=hf[:, fc, ncc * P:(ncc + 1) * P],
                                rhs=w2f[:, fc, dtile * 512:(dtile + 1) * 512],
                                start=(fc == 0),
                                stop=(fc == DC - 1),
                            )
                        ysl = y_acc[:, ncc, dtile * 512:(dtile + 1) * 512]
                        if f == 0:
                            # y = psum + x2 (residual folded into accumulator)
                            nc.vector.scalar_tensor_tensor(
                                out=ysl, in0=ps, scalar=1.0,
                                in1=x2_sb[:, ncc, dtile * 512:(dtile + 1) * 512],
                                op0=ALU.mult, op1=ALU.add,
                            )
                        else:
                            nc.vector.tensor_tensor(
                                out=ysl, in0=ps, in1=ysl, op=ALU.add
                            )

            # ---- LN3 -> out ----
            layernorm(y_acc, None, g3_d, be3_d, None, lnp3, final=True)

        for _rep in range(loop_n):
            one_layer()

    nc.compile()
    _BUILD_CACHE[key] = nc
    return nc


def _prep_inputs(inputs):
    """Host-side shard prep: transposes, bf16 casts, bias folding.
    Returns (in_maps, ln_identity)."""
    f32 = np.float32

    def t_bf16(a):
        return np.ascontiguousarray(np.asarray(a, dtype=f32).T).astype(BF16)

    x = np.asarray(inputs["x"], dtype=f32)
    memory = np.asarray(inputs["memory"], dtype=f32)
    Wq2 = np.asarray(inputs["Wq2"], dtype=f32)
    W1 = np.asarray(inputs["W1"], dtype=f32)
    bq2 = np.asarray(inputs["bq2"], dtype=f32)
    bv1 = np.asarray(inputs["bv1"], dtype=f32)
    bv2 = np.asarray(inputs["bv2"], dtype=f32)
    b1 = np.asarray(inputs["b1"], dtype=f32)
    b2 = np.asarray(inputs["b2"], dtype=f32)
    g1 = np.asarray(inputs["g1"], dtype=f32)
    be1 = np.asarray(inputs["be1"], dtype=f32)
    g2 = np.asarray(inputs["g2"], dtype=f32)
    be2 = np.asarray(inputs["be2"], dtype=f32)
    g3 = np.asarray(inputs["g3"], dtype=f32)
    be3 = np.asarray(inputs["be3"], dtype=f32)

    be1p = (be1 + bv2).astype(f32)
    be2p = (be2 + b2).astype(f32)
    ln_identity = bool(
        np.all(g1 == 1) and np.all(g2 == 1) and np.all(g3 == 1)
        and np.all(be1p == 0) and np.all(be2p == 0) and np.all(be3 == 0)
    )

    shared = {
        "WqT1": t_bf16(inputs["Wq1"]), "WkT1": t_bf16(inputs["Wk1"]),
        "WvT1": t_bf16(inputs["Wv1"]), "WqT2": t_bf16(Wq2),
        "WkT2": t_bf16(inputs["Wk2"]), "WvT2": t_bf16(inputs["Wv2"]),
        "W1T": t_bf16(W1), "W2T": t_bf16(inputs["W2"]),
        "bq1": np.asarray(inputs["bq1"], f32),
        "bk1": np.asarray(inputs["bk1"], f32),
        "bq2p": (bq2 - Wq2 @ bv2).astype(f32),
        "bk2": np.asarray(inputs["bk2"], f32),
        "b1p": (b1 - W1 @ b2).astype(f32),
        "g1": g1.astype(BF16), "be1p": be1p.astype(BF16),
        "g2": g2.astype(BF16), "be2p": be2p.astype(BF16),
        "g3": g3.astype(BF16), "be3": be3.astype(BF16),
    }

    in_maps = []
    for i in range(N_CORES):
        m = dict(shared)
        m["x_tok"] = (x[i] + bv1[None, :]).astype(BF16)
        m["xT"] = t_bf16(x[i])
        m["memT"] = t_bf16(memory[i])
        in_maps.append(m)
    return in_maps, ln_identity


def kernel(**inputs) -> np.ndarray:
    os.environ.setdefault("MYCRO_LOCAL_CACHE", "1")
    from concourse.bass_utils import run_bass_kernel_spmd

    in_maps, ln_identity = _prep_inputs(inputs)
    nc = _build_program(1, ln_identity)
    res = run_bass_kernel_spmd(nc, in_maps, core_ids=list(range(N_CORES)))
    out = np.stack([res.results[i]["out"] for i in range(N_CORES)], axis=0)
    return out.astype(np.float32)


# revision 23
# speedup vs baseline: 2.6063x; 2.6063x over previous
"""Trainium2 Bass kernel for nn_DecoderLayer (post-LN decoder layer).

Sharding: data-parallel over batch. B=8 batch elements -> 8 NeuronCores,
one full decoder layer per core, zero collectives.

Per-core layout strategy:
  - Matmul inputs are feature-major (D on partitions): xT, memT, WT (host
    pre-transposed, bf16).
  - Residual stream + LayerNorm are token-major (tokens on partitions) so
    LN reductions run along the free dim and softmax-normalization is a
    per-partition scalar op.
  - Attention per head h: scoresT[m,n] = k_h^T q_h (lhsT=kT_h, K=64),
    exp on ScalarE (scale=1/8 fused, no max subtraction -- scores are O(1)
    here), then o[n,:] = lhsT(expT chunk)^T @ [v_h | ones]; column 64 gives
    sum(exp) so normalization is one reciprocal + per-partition multiply
    on VectorE. ScalarE runs *only* exp (+2 small LN ops): all PSUM
    evictions are pinned to VectorE.
  - LayerNorm is two-pass: residual+bn_stats per chunk, then one batched
    Ln/Exp pair for all 8 chunks' rstd (avoids ACT table-set thrashing),
    then the apply pass. x1/x2 stay resident in SBUF; the feature-major
    copies are made with 64 SBUF->SBUF xbar DMA transposes.
  - FFN is chunked over FF (4 x 1024) with an SBUF f32 accumulator that is
    initialized with the residual input, so LN3 reads it directly.
  - Host folds biases: x_tok = x + bv1; be1' = be1 + bv2; bq2' = bq2 -
    Wq2 @ bv2; be2' = be2 + b2; b1' = b1 - W1 @ b2.  bq/bk/b1 are applied
    on-device as per-partition bias during PSUM eviction. When g==1 and
    be'==0 (checked at runtime) a variant without the g/be multiplies is
    used.
"""

import os
import numpy as np
import ml_dtypes

BF16 = ml_dtypes.bfloat16

D = 1024
N = 1024
H = 16
DK = 64
FF = 4096
P = 128
DC = D // P     # 8 feature chunks
NC_ = N // P    # 8 token chunks
EPS = 1e-5
N_CORES = 8

_BUILD_CACHE = {}


def _build_program(loop_n=1, ln_identity=False):
    """Build the Bass/Tile program. loop_n > 1 replicates the layer body
    (timing only). ln_identity skips the LN gamma/beta applies."""
    key = (loop_n, ln_identity)
    if key in _BUILD_CACHE:
        return _BUILD_CACHE[key]

    from contextlib import ExitStack

    import concourse.bass as bass
    import concourse.mybir as mybir
    import concourse.tile as tile
    from concourse import bacc
    from concourse.masks import make_identity

    dt = mybir.dt
    AF = mybir.ActivationFunctionType
    ALU = mybir.AluOpType

    nc = bacc.Bacc("TRN2", target_bir_lowering=False, debug=False)

    # ---- DRAM parameters (per core) ----
    x_tok_d = nc.dram_tensor("x_tok", [N, D], dt.bfloat16, kind="ExternalInput")
    xT_d = nc.dram_tensor("xT", [D, N], dt.bfloat16, kind="ExternalInput")
    memT_d = nc.dram_tensor("memT", [D, N], dt.bfloat16, kind="ExternalInput")
    WqT1_d = nc.dram_tensor("WqT1", [D, D], dt.bfloat16, kind="ExternalInput")
    WkT1_d = nc.dram_tensor("WkT1", [D, D], dt.bfloat16, kind="ExternalInput")
    WvT1_d = nc.dram_tensor("WvT1", [D, D], dt.bfloat16, kind="ExternalInput")
    WqT2_d = nc.dram_tensor("WqT2", [D, D], dt.bfloat16, kind="ExternalInput")
    WkT2_d = nc.dram_tensor("WkT2", [D, D], dt.bfloat16, kind="ExternalInput")
    WvT2_d = nc.dram_tensor("WvT2", [D, D], dt.bfloat16, kind="ExternalInput")
    W1T_d = nc.dram_tensor("W1T", [D, FF], dt.bfloat16, kind="ExternalInput")
    W2T_d = nc.dram_tensor("W2T", [FF, D], dt.bfloat16, kind="ExternalInput")
    bq1_d = nc.dram_tensor("bq1", [D], dt.float32, kind="ExternalInput")
    bk1_d = nc.dram_tensor("bk1", [D], dt.float32, kind="ExternalInput")
    bq2_d = nc.dram_tensor("bq2p", [D], dt.float32, kind="ExternalInput")
    bk2_d = nc.dram_tensor("bk2", [D], dt.float32, kind="ExternalInput")
    b1_d = nc.dram_tensor("b1p", [FF], dt.float32, kind="ExternalInput")
    g1_d = nc.dram_tensor("g1", [D], dt.bfloat16, kind="ExternalInput")
    be1_d = nc.dram_tensor("be1p", [D], dt.bfloat16, kind="ExternalInput")
    g2_d = nc.dram_tensor("g2", [D], dt.bfloat16, kind="ExternalInput")
    be2_d = nc.dram_tensor("be2p", [D], dt.bfloat16, kind="ExternalInput")
    g3_d = nc.dram_tensor("g3", [D], dt.bfloat16, kind="ExternalInput")
    be3_d = nc.dram_tensor("be3", [D], dt.bfloat16, kind="ExternalInput")
    out_d = nc.dram_tensor("out", [N, D], dt.float32, kind="ExternalOutput")

    def bcast_ap(handle, n):
        return bass.AP(tensor=handle, offset=0, ap=[[0, P], [1, n]])

    def colmajor_ap(handle, chunks):
        # [chunks*P] f32 vector -> SBUF [P, chunks] with [p, c] = v[c*P + p]
        return bass.AP(tensor=handle, offset=0, ap=[[1, P], [P, chunks]])

    with tile.TileContext(nc) as tc, ExitStack() as top:
        consts = top.enter_context(tc.tile_pool(name="consts", bufs=1))
        wpool = top.enter_context(tc.tile_pool(name="w", bufs=2))
        xTp = top.enter_context(tc.tile_pool(name="xT", bufs=1))
        opool = top.enter_context(tc.tile_pool(name="o", bufs=1))
        xsb = top.enter_context(tc.tile_pool(name="xsb", bufs=2))
        small = top.enter_context(tc.tile_pool(name="small", bufs=10))

        # ---- constants ----
        bq1_sb = consts.tile([P, DC], dt.float32, tag="c_bq1")
        nc.sync.dma_start(out=bq1_sb, in_=colmajor_ap(bq1_d, DC))
        bk1_sb = consts.tile([P, DC], dt.float32, tag="c_bk1")
        nc.sync.dma_start(out=bk1_sb, in_=colmajor_ap(bk1_d, DC))
        bq2_sb = consts.tile([P, DC], dt.float32, tag="c_bq2")
        nc.sync.dma_start(out=bq2_sb, in_=colmajor_ap(bq2_d, DC))
        bk2_sb = consts.tile([P, DC], dt.float32, tag="c_bk2")
        nc.sync.dma_start(out=bk2_sb, in_=colmajor_ap(bk2_d, DC))
        b1_sb = consts.tile([P, FF // P], dt.float32, tag="c_b1")
        nc.sync.dma_start(out=b1_sb, in_=colmajor_ap(b1_d, FF // P))
        eps_sb = consts.tile([P, 1], dt.float32, tag="c_eps")
        nc.vector.memset(eps_sb, EPS)
        ident_sb = consts.tile([P, P], dt.bfloat16, tag="c_ident")
        make_identity(nc, ident_sb)

        def load_w(dram_h, row0=0, nrows=None, col0=0, ncolw=D):
            """Stream rows [row0, row0+nrows) cols [col0, col0+ncolw) of a
            transposed weight into SBUF [P, nrows//P, ncolw], split per
            k-chunk so consuming matmuls start as soon as chunk 0 lands."""
            rows, ncols = dram_h.shape
            if nrows is None:
                nrows = rows
            w = wpool.tile([P, nrows // P, ncolw], dt.bfloat16, tag="w",
                           name=f"w_{dram_h.name}_{row0}_{col0}")
            for kc in range(nrows // P):
                nc.sync.dma_start(
                    out=w[:, kc, :],
                    in_=bass.AP(tensor=dram_h,
                                offset=(row0 + kc * P) * ncols + col0,
                                ap=[[ncols, P], [1, ncolw]]),
                )
            return w

        def proj_fm(out_sb, wT_sb, inT_sb, bias_sb, pp):
            """Feature-major projection: out[o,n] = sum_d WT[d,o] inT[d,n] + b[o]."""
            for dc in range(DC):
                for nt in range(2):
                    ps = pp.tile([P, 512], dt.float32, tag="pp")
                    for kc in range(DC):
                        nc.tensor.matmul(
                            ps,
                            lhsT=wT_sb[:, kc, dc * P:(dc + 1) * P],
                            rhs=inT_sb[:, kc, nt * 512:(nt + 1) * 512],
                            start=(kc == 0),
                            stop=(kc == DC - 1),
                        )
                    nc.vector.tensor_scalar(
                        out_sb[:, dc, nt * 512:(nt + 1) * 512],
                        ps, bias_sb[:, dc:dc + 1], None, ALU.add,
                    )

        def proj_v65(v65_sb, wT_sb, inT_sb, pp):
            """Token-major V projection into [P, mc, h, 0:64]; col 64 stays 1.0."""
            nc.gpsimd.memset(v65_sb, 1.0)
            for mc in range(NC_):
                for ot in range(2):
                    ps = pp.tile([P, 512], dt.float32, tag="pp")
                    for kc in range(DC):
                        nc.tensor.matmul(
                            ps,
                            lhsT=inT_sb[:, kc, mc * P:(mc + 1) * P],
                            rhs=wT_sb[:, kc, ot * 512:(ot + 1) * 512],
                            start=(kc == 0),
                            stop=(kc == DC - 1),
                        )
                    nc.vector.tensor_copy(
                        v65_sb[:, mc, ot * 8:(ot + 1) * 8, 0:64],
                        ps.rearrange("p (h e) -> p h e", h=8),
                    )

        def attention(qT_sb, kT_sb, v65_sb, o_sb, ep, sp, op65):
            for h_idx in range(H):
                hp, half = divmod(h_idx, 2)
                lo = half * 64
                expT = ep.tile([P, NC_, N], dt.bfloat16, tag="exp",
                               name=f"exp_{h_idx}")
                for mc in range(NC_):
                    ps = sp.tile([P, N], dt.float32, tag="sp",
                                 name=f"ps_{h_idx}_{mc}")
                    for nt in range(2):
                        nc.tensor.matmul(
                            ps[:, nt * 512:(nt + 1) * 512],
                            lhsT=kT_sb[lo:lo + 64, hp, mc * P:(mc + 1) * P],
                            rhs=qT_sb[lo:lo + 64, hp, nt * 512:(nt + 1) * 512],
                            start=True, stop=True,
                        )
                    nc.scalar.activation(
                        expT[:, mc, :], ps, AF.Exp, scale=0.125,
                    )
                for ncc in range(NC_):
                    po = op65.tile([P, 65], dt.float32, tag="op65")
                    for mc in range(NC_):
                        nc.tensor.matmul(
                            po,
                            lhsT=expT[:, mc, ncc * P:(ncc + 1) * P],
                            rhs=v65_sb[:, mc, h_idx, :],
                            start=(mc == 0),
                            stop=(mc == NC_ - 1),
                        )
                    rec = small.tile([P, 1], dt.float32, tag="rec")
                    nc.vector.reciprocal(rec, po[:, 64:65])
                    nc.vector.tensor_scalar(
                        o_sb[:, ncc, h_idx * 64:(h_idx + 1) * 64],
                        po[:, 0:64], rec, None, ALU.mult,
                    )

        def layernorm(r_sb, x_src, g_d, be_d, dst_sb, lnp, final=False):
            """dst = LN(x_src + r_sb)*g + be, token-major, two passes.

            r_sb: SBUF [P, NC_, D] (attention out / f32 FFN accumulator);
                  mutated in place to hold the residual sum unless x_src
                  is None.
            x_src: DRAM handle (LN1), SBUF [P, NC_, D] tensor (LN2), or
                  None (LN3: r_sb already holds the residual sum).
            dst_sb: SBUF [P, NC_, D] bf16 (x1/x2) or, if final, the DRAM
                  output handle.
            """
            mv = small.tile([P, NC_, 2], dt.float32, tag="mv",
                            name=f"mv_{'f' if final else dst_sb.name}")
            for ncc in range(NC_):
                if x_src is not None:
                    if callable(getattr(x_src, "ap", None)):  # DRAM handle
                        xin = lnp.tile([P, D], dt.bfloat16, tag="lnx")
                        nc.sync.dma_start(
                            out=xin, in_=x_src.ap()[ncc * P:(ncc + 1) * P, :]
                        )
                    else:
                        xin = x_src[:, ncc, :]
                    r = r_sb[:, ncc, :]
                    nc.vector.tensor_tensor(out=r, in0=r, in1=xin, op=ALU.add)
                else:
                    r = r_sb[:, ncc, :]
                stats = small.tile([P, 2, 6], dt.float32, tag="stats")
                nc.vector.bn_stats(stats[:, 0, :], r[:, 0:512])
                nc.vector.bn_stats(stats[:, 1, :], r[:, 512:1024])
                nc.vector.bn_aggr(mv[:, ncc, :], stats)
            # batched rstd = exp(-0.5*ln(var+eps)): one table-load pair per LN
            lnv = small.tile([P, NC_], dt.float32, tag="lnv")
            nc.scalar.activation(lnv, mv[:, :, 1], AF.Ln, bias=eps_sb)
            rstd = small.tile([P, NC_], dt.float32, tag="rstd")
            nc.scalar.activation(rstd, lnv, AF.Exp, scale=-0.5)
            nmr = small.tile([P, NC_], dt.float32, tag="nmr")
            nc.vector.tensor_tensor(out=nmr, in0=mv[:, :, 0], in1=rstd,
                                    op=ALU.mult)
            nc.vector.tensor_scalar(nmr, nmr, -1.0, None, ALU.mult)
            if not ln_identity:
                g_t = lnp.tile([P, D], dt.bfloat16, tag="lng",
                               name=f"g_{g_d.name}")
                nc.sync.dma_start(out=g_t, in_=bcast_ap(g_d, D))
                be_t = lnp.tile([P, D], dt.bfloat16, tag="lnbe",
                                name=f"be_{be_d.name}")
                nc.sync.dma_start(out=be_t, in_=bcast_ap(be_d, D))
            for ncc in range(NC_):
                r = r_sb[:, ncc, :]
                if final:
                    oc = lnp.tile([P, D], dt.float32, tag="lnof")
                    tgt = oc
                else:
                    tgt = dst_sb[:, ncc, :]
                if ln_identity:
                    nc.any.tensor_scalar(
                        tgt, r, rstd[:, ncc:ncc + 1], nmr[:, ncc:ncc + 1],
                        ALU.mult, ALU.add)
                else:
                    t = lnp.tile([P, D], dt.bfloat16, tag="lnt")
                    nc.vector.tensor_scalar(
                        t, r, rstd[:, ncc:ncc + 1], nmr[:, ncc:ncc + 1],
                        ALU.mult, ALU.add)
                    nc.vector.tensor_tensor(out=t, in0=t, in1=g_t, op=ALU.mult)
                    nc.vector.tensor_tensor(out=tgt, in0=t, in1=be_t,
                                            op=ALU.add)
                if final:
                    nc.sync.dma_start(
                        out=out_d.ap()[ncc * P:(ncc + 1) * P, :], in_=tgt
                    )

        def transpose_in(xT_sb, src_sb):
            """64 PE transposes + DVE evictions (PE is idle in LN windows)."""
            with tc.tile_pool(name="tp", bufs=4, space="PSUM") as tpp:
                for ncc in range(NC_):
                    for dc in range(DC):
                        pst = tpp.tile([P, P], dt.bfloat16, tag="tp")
                        nc.tensor.transpose(
                            pst, src_sb[:, ncc, dc * P:(dc + 1) * P], ident_sb)
                        nc.any.tensor_copy(
                            xT_sb[:, dc, ncc * P:(ncc + 1) * P], pst)

        def one_layer():
         with ExitStack() as attn_scope:
            qp = attn_scope.enter_context(tc.tile_pool(name="q", bufs=1))
            kp = attn_scope.enter_context(tc.tile_pool(name="k", bufs=1))
            vp = attn_scope.enter_context(tc.tile_pool(name="v", bufs=1))
            ep = attn_scope.enter_context(tc.tile_pool(name="exp", bufs=2))

            with tc.tile_pool(name="mem", bufs=1) as memp:
                xT0 = xTp.tile([P, DC, N], dt.bfloat16, tag="xT")
                for kc in range(DC):
                    nc.sync.dma_start(
                        out=xT0[:, kc, :],
                        in_=bass.AP(tensor=xT_d, offset=kc * P * N,
                                    ap=[[N, P], [1, N]]))
                # ---- self-attention QKV ----
                q1T = qp.tile([P, DC, N], dt.bfloat16, tag="qT")
                k1T = kp.tile([P, DC, N], dt.bfloat16, tag="kT")
                v1 = vp.tile([P, NC_, H, 65], dt.bfloat16, tag="v65")
                with tc.tile_pool(name="pp1", bufs=4, space="PSUM") as pp:
                    proj_fm(q1T, load_w(WqT1_d), xT0, bq1_sb, pp)
                    proj_fm(k1T, load_w(WkT1_d), xT0, bk1_sb, pp)
                    proj_v65(v1, load_w(WvT1_d), xT0, pp)

                # memory stream loads late: first needed at the K2/V2 projs
                memT = memp.tile([P, DC, N], dt.bfloat16, tag="memT")
                for kc in range(DC):
                    nc.sync.dma_start(
                        out=memT[:, kc, :],
                        in_=bass.AP(tensor=memT_d, offset=kc * P * N,
                                    ap=[[N, P], [1, N]]))

                # ---- self-attention ----
                o1 = opool.tile([P, NC_, D], dt.bfloat16, tag="o")
                with tc.tile_pool(name="sp1", bufs=3, space="PSUM") as sp, \
                     tc.tile_pool(name="ov1", bufs=2, space="PSUM") as op65:
                    attention(q1T, k1T, v1, o1, ep, sp, op65)

                # ---- cross-attention K/V (independent of LN1: fills the
                # LN1 gap with PE work) ----
                k2T = kp.tile([P, DC, N], dt.bfloat16, tag="kT")
                v2 = vp.tile([P, NC_, H, 65], dt.bfloat16, tag="v65")
                with tc.tile_pool(name="pp2", bufs=4, space="PSUM") as pp:
                    proj_fm(k2T, load_w(WkT2_d), memT, bk2_sb, pp)
                    proj_v65(v2, load_w(WvT2_d), memT, pp)

            # ---- LN1 -> x1 (SBUF resident) + x1T ----
            x1_sb = xsb.tile([P, NC_, D], dt.bfloat16, tag="xsb", name="x1sb")
            with tc.tile_pool(name="ln1", bufs=2) as lnp:
                layernorm(o1, x_tok_d, g1_d, be1_d, x1_sb, lnp)
            x1T = xTp.tile([P, DC, N], dt.bfloat16, tag="xT")
            transpose_in(x1T, x1_sb)

            # ---- cross-attention Q ----
            q2T = qp.tile([P, DC, N], dt.bfloat16, tag="qT")
            with tc.tile_pool(name="pp3", bufs=4, space="PSUM") as pp:
                proj_fm(q2T, load_w(WqT2_d), x1T, bq2_sb, pp)

            # ---- cross-attention ----
            o2 = opool.tile([P, NC_, D], dt.bfloat16, tag="o")
            with tc.tile_pool(name="sp2", bufs=3, space="PSUM") as sp, \
                 tc.tile_pool(name="ov2", bufs=2, space="PSUM") as op65:
                attention(q2T, k2T, v2, o2, ep, sp, op65)

            # ---- LN2 -> x2 (SBUF resident) + x2T ----
            x2_sb = xsb.tile([P, NC_, D], dt.bfloat16, tag="xsb", name="x2sb")
            with tc.tile_pool(name="ln2", bufs=2) as lnp:
                layernorm(o2, x1_sb, g2_d, be2_d, x2_sb, lnp)
            x2T = xTp.tile([P, DC, N], dt.bfloat16, tag="xT")
            transpose_in(x2T, x2_sb)

         # ---- FFN ----
         with ExitStack() as ffn_scope:
            hp_ = ffn_scope.enter_context(tc.tile_pool(name="h", bufs=2))
            yp = ffn_scope.enter_context(tc.tile_pool(name="y", bufs=1))
            lnp3 = ffn_scope.enter_context(tc.tile_pool(name="ln3", bufs=2))
            ppf = ffn_scope.enter_context(
                tc.tile_pool(name="ppf", bufs=4, space="PSUM"))
            y_acc = yp.tile([P, NC_, D], dt.float32, tag="y")
            for f in range(FF // D):
                w1f = load_w(W1T_d, col0=f * D)
                hf = hp_.tile([P, DC, N], dt.bfloat16, tag="h")
                for fc in range(DC):
                    for nt in range(2):
                        ps = ppf.tile([P, 512], dt.float32, tag="pp")
                        for kc in range(DC):
                            nc.tensor.matmul(
                                ps,
                                lhsT=w1f[:, kc, fc * P:(fc + 1) * P],
                                rhs=x2T[:, kc, nt * 512:(nt + 1) * 512],
                                start=(kc == 0),
                                stop=(kc == DC - 1),
                            )
                        # h = relu(psum + b1')
                        nc.vector.tensor_scalar(
                            hf[:, fc, nt * 512:(nt + 1) * 512],
                            ps, b1_sb[:, f * DC + fc:f * DC + fc + 1], 0.0,
                            ALU.add, ALU.max,
                        )
                w2f = load_w(W2T_d, row0=f * D, nrows=D)
                for ncc in range(NC_):
                    for dtile in range(2):
                        ps = ppf.tile([P, 512], dt.float32, tag="pp")
                        for fc in range(DC):
                            nc.tensor.matmul(
                                ps,
                                lhsT=hf[:, fc, ncc * P:(ncc + 1) * P],
                                rhs=w2f[:, fc, dtile * 512:(dtile + 1) * 512],
                                start=(fc == 0),
                                stop=(fc == DC - 1),
                            )
                        ysl = y_acc[:, ncc, dtile * 512:(dtile + 1) * 512]
                        if f == 0:
                            # y = psum + x2 (residual folded into accumulator)
                            nc.vector.scalar_tensor_tensor(
                                out=ysl, in0=ps, scalar=1.0,
                                in1=x2_sb[:, ncc, dtile * 512:(dtile + 1) * 512],
                                op0=ALU.mult, op1=ALU.add,
                            )
                        else:
                            nc.vector.tensor_tensor(
                                out=ysl, in0=ps, in1=ysl, op=ALU.add
                            )

            # ---- LN3 -> out ----
            layernorm(y_acc, None, g3_d, be3_d, None, lnp3, final=True)

        for _rep in range(loop_n):
            one_layer()

    nc.compile()
    _BUILD_CACHE[key] = nc
    return nc


def _prep_inputs(inputs):
    """Host-side shard prep: transposes, bf16 casts, bias folding.
    Returns (in_maps, ln_identity)."""
    f32 = np.float32

    def t_bf16(a):
        return np.ascontiguousarray(np.asarray(a, dtype=f32).T).astype(BF16)

    x = np.asarray(inputs["x"], dtype=f32)
    memory = np.asarray(inputs["memory"], dtype=f32)
    Wq2 = np.asarray(inputs["Wq2"], dtype=f32)
    W1 = np.asarray(inputs["W1"], dtype=f32)
    bq2 = np.asarray(inputs["bq2"], dtype=f32)
    bv1 = np.asarray(inputs["bv1"], dtype=f32)
    bv2 = np.asarray(inputs["bv2"], dtype=f32)
    b1 = np.asarray(inputs["b1"], dtype=f32)
    b2 = np.asarray(inputs["b2"], dtype=f32)
    g1 = np.asarray(inputs["g1"], dtype=f32)
    be1 = np.asarray(inputs["be1"], dtype=f32)
    g2 = np.asarray(inputs["g2"], dtype=f32)
    be2 = np.asarray(inputs["be2"], dtype=f32)
    g3 = np.asarray(inputs["g3"], dtype=f32)
    be3 = np.asarray(inputs["be3"], dtype=f32)

    be1p = (be1 + bv2).astype(f32)
    be2p = (be2 + b2).astype(f32)
    ln_identity = bool(
        np.all(g1 == 1) and np.all(g2 == 1) and np.all(g3 == 1)
        and np.all(be1p == 0) and np.all(be2p == 0) and np.all(be3 == 0)
    )

    shared = {
        "WqT1": t_bf16(inputs["Wq1"]), "WkT1": t_bf16(inputs["Wk1"]),
        "WvT1": t_bf16(inputs["Wv1"]), "WqT2": t_bf16(Wq2),
        "WkT2": t_bf16(inputs["Wk2"]), "WvT2": t_bf16(inputs["Wv2"]),
        "W1T": t_bf16(W1), "W2T": t_bf16(inputs["W2"]),
        "bq1": np.asarray(inputs["bq1"], f32),
        "bk1": np.asarray(inputs["bk1"], f32),
        "bq2p": (bq2 - Wq2 @ bv2).astype(f32),
        "bk2": np.asarray(inputs["bk2"], f32),
        "b1p": (b1 - W1 @ b2).astype(f32),
        "g1": g1.astype(BF16), "be1p": be1p.astype(BF16),
        "g2": g2.astype(BF16), "be2p": be2p.astype(BF16),
        "g3": g3.astype(BF16), "be3": be3.astype(BF16),
    }

    in_maps = []
    for i in range(N_CORES):
        m = dict(shared)
        m["x_tok"] = (x[i] + bv1[None, :]).astype(BF16)
        m["xT"] = t_bf16(x[i])
        m["memT"] = t_bf16(memory[i])
        in_maps.append(m)
    return in_maps, ln_identity


def kernel(**inputs) -> np.ndarray:
    os.environ.setdefault("MYCRO_LOCAL_CACHE", "1")
    from concourse.bass_utils import run_bass_kernel_spmd

    in_maps, ln_identity = _prep_inputs(inputs)
    nc = _build_program(1, ln_identity)
    res = run_bass_kernel_spmd(nc, in_maps, core_ids=list(range(N_CORES)))
    out = np.stack([res.results[i]["out"] for i in range(N_CORES)], axis=0)
    return out.astype(np.float32)
